# revision 1
# baseline (speedup 1.0000x reference)
"""DN4 retrieval-KNN kernel for Trainium2 (8 NeuronCores, SPMD).

Computation (per episode batch b):
  sup   = mean_k support[b]  -> (5, 64, 441)           (class prototypes, local descriptors)
  logits[q, w] = sum_m max_n <qn[q,:,m], sn[w,:,n]>    (cosine sims of l2-normalized descriptors)

Sharding: 4 cores per batch element, 19 queries per core (75 = 19+19+19+18, last
core padded).  Support is replicated per batch-group; no cross-core comms.

Device algorithm (per core):
  - support: for each class, PE transpose-accumulate the 5 shots into (m,c) layout
    (the /5 of the mean cancels under l2-normalization), per-partition sumsq ->
    1/sqrt -> scale -> cast bf16 -> PE transpose back to (c, m); replicate to
    partitions 64..127 so matmuls can be packed 2x into PE row groups (K=64).
  - queries: q zero-padded to 512 cols; sim[mchunk(128), n(441)] = qb^T @ sn via
    bf16 matmuls alternating row groups (0,0)/(64,0); DVE reduce_max over n;
    maxv scaled by 1/||q_m|| (exact: positive per-row scale commutes with max);
    final sum over m via one PE matmul against a ones vector.
"""

import numpy as np

import concourse.bacc as bacc
import concourse.bass as bass
import concourse.mybir as mybir
import concourse.tile as tile
from concourse.bass_utils import run_bass_kernel_spmd

F32 = mybir.dt.float32
BF16 = mybir.dt.bfloat16
AX = mybir.AxisListType
ALU = mybir.AluOpType
ACT_SQRT = mybir.ActivationFunctionType.Sqrt

B, NWAY, KSHOT, Q, C, HW = 2, 5, 5, 75, 64, 441  # 21*21 = 441
QPC = 19          # queries per core (8 cores: 4 per batch, 19/19/19/18+pad)
PADW = 512        # query free dim padded so m-chunks are 4x128 exactly
NCHUNK = 4
NPAIR_DVE = 3     # chunks whose classes (0,1) are pair-merged + reduced on DVE
USE_TTR = False   # ACT-evac + DVE tensor_tensor_reduce path (needs TTR on HW)
EPS = 1e-6        # added under sqrt; ssq ~ 64 for real data, pads give finite invn

_CACHE = {}


def _chunk_cols(j):
    lo = j * 128
    hi = min(lo + 128, HW)
    return lo, hi


def _build_program():
    nc = bacc.Bacc("TRN2", target_bir_lowering=False, debug=False, num_devices=8)

    sup_d = nc.dram_tensor("sup", [NWAY * KSHOT, C, HW], F32, kind="ExternalInput").ap()
    qry_d = nc.dram_tensor("qry", [QPC, C, HW], F32, kind="ExternalInput").ap()
    idn_d = nc.dram_tensor("idn", [128, 128], F32, kind="ExternalInput").ap()
    out_d = nc.dram_tensor("out", [QPC, NWAY], F32, kind="ExternalOutput").ap()

    with tile.TileContext(nc) as tc:
        with tc.tile_pool(name="const", bufs=1) as cpool:
            ident = cpool.tile([128, 128], F32)
            nc.sync.dma_start(ident[:], idn_d[:])
            identb = cpool.tile([128, 128], BF16)
            nc.vector.tensor_copy(identb[:], ident[:])
            ones = cpool.tile([128, 1], F32)
            nc.vector.memset(ones[:], 1.0)
            eps = cpool.tile([128, 1], F32)
            nc.vector.memset(eps[:], EPS)
            sn = [cpool.tile([128, HW], BF16, name=f"sn{w}") for w in range(NWAY)]
            stage = cpool.tile([NWAY, QPC], F32)

            # ---------------- support prototypes ----------------
            with (
                tc.tile_pool(name="sup_sb", bufs=2) as spool,
                tc.tile_pool(name="sup_ps", bufs=2, space="PSUM") as sps,
            ):
                for w in range(NWAY):
                    s5 = spool.tile([C, KSHOT * HW], F32, tag="s5")
                    nc.sync.dma_start(
                        s5[:].rearrange("c (k m) -> c k m", k=KSHOT),
                        sup_d[w * KSHOT : (w + 1) * KSHOT].rearrange("k c m -> c k m"),
                    )
                    for j in range(NCHUNK):
                        lo, hi = _chunk_cols(j)
                        wj = hi - lo
                        # sum of shots, transposed into (m, c): PSUM accumulation
                        sT = sps.tile([128, C], F32, tag="sT")
                        for k in range(KSHOT):
                            nc.tensor.matmul(
                                sT[0:wj, :],
                                lhsT=s5[:, k * HW + lo : k * HW + hi],
                                rhs=ident[0:C, 0:C],
                                is_transpose=True,
                                start=(k == 0),
                                stop=(k == KSHOT - 1),
                            )
                        sq = spool.tile([128, C], F32, tag="sq")
                        nc.scalar.square(sq[0:wj, :], sT[0:wj, :])
                        ssq = spool.tile([128, 1], F32, tag="ssq")
                        nc.vector.reduce_sum(ssq[0:wj, :], sq[0:wj, :], axis=AX.X)
                        sst = spool.tile([128, 1], F32, tag="sst")
                        nc.scalar.activation(
                            sst[0:wj, :], ssq[0:wj, :], ACT_SQRT, bias=eps[0:wj, :]
                        )
                        inv = spool.tile([128, 1], F32, tag="inv")
                        nc.vector.reciprocal(inv[0:wj, :], sst[0:wj, :])
                        snT = spool.tile([128, C], BF16, tag="snT")
                        nc.vector.tensor_scalar_mul(snT[0:wj, :], sT[0:wj, :], inv[0:wj, :])
                        # transpose back to (c, m) bf16
                        snb = sps.tile([C, 128], BF16, tag="snb")
                        nc.tensor.matmul(
                            snb[:, 0:wj],
                            lhsT=snT[0:wj, :],
                            rhs=identb[0:wj, 0:wj],
                            is_transpose=True,
                            start=True,
                            stop=True,
                        )
                        nc.scalar.copy(sn[w][0:C, lo:hi], snb[:, 0:wj])
                    # replicate to partitions 64..127 for row-group packing
                    nc.sync.dma_start(sn[w][C:128, :], sn[w][0:C, :])

            # ---------------- queries ----------------
            # Max-reduction of the 20 sim tiles per query is split across three
            # paths to use every engine: DVE reduces pair-merged PSUM tiles
            # directly; ACT evacuates the rest to SBUF where GpSimd reduces.
            with (
                tc.tile_pool(name="q_sb", bufs=3) as qpool,
                tc.tile_pool(name="q_small", bufs=3) as qsm,
                tc.tile_pool(name="q_ps", bufs=1, space="PSUM") as qps,
                tc.tile_pool(name="pair_ps", bufs=2 if USE_TTR else 1, space="PSUM") as pairps,
                tc.tile_pool(name="sing_ps", bufs=2 if USE_TTR else 1, space="PSUM") as singps,
                tc.tile_pool(name="log_ps", bufs=1, space="PSUM") as logps,
            ):
                for i in range(QPC):
                    q2 = qpool.tile([128, PADW], F32, tag="q2")
                    nc.gpsimd.memset(q2[:, HW:PADW], 0.0)
                    nc.sync.dma_start(q2[0:C, 0:HW], qry_d[i])
                    nc.sync.dma_start(q2[C:128, 0:HW], qry_d[i])
                    qb = qpool.tile([128, PADW], BF16, tag="qb")
                    nc.gpsimd.dma_start(qb[:], q2[:])  # SWDGE cast f32->bf16

                    # 1/||q_m||: transpose raw fp32 (padded), square, rowsum, rsqrt
                    qT = qps.tile([128, NCHUNK * C], F32, tag="qT")
                    for j in range(NCHUNK):
                        nc.tensor.matmul(
                            qT[:, j * C : (j + 1) * C],
                            lhsT=q2[0:C, j * 128 : (j + 1) * 128],
                            rhs=ident[0:C, 0:C],
                            is_transpose=True,
                            start=True,
                            stop=True,
                        )
                    sqv = qpool.tile([128, NCHUNK * C], F32, tag="sqv")
                    nc.scalar.square(sqv[:], qT[:])
                    ssq = qsm.tile([128, NCHUNK], F32, tag="qssq")
                    nc.vector.reduce_sum(
                        ssq[:], sqv[:].rearrange("p (j c) -> p j c", j=NCHUNK), axis=AX.X
                    )
                    sst = qsm.tile([128, NCHUNK], F32, tag="qsst")
                    nc.scalar.activation(sst[:], ssq[:], ACT_SQRT, bias=eps[:])
                    invq = qsm.tile([128, NCHUNK], F32, tag="invq")
                    nc.vector.reciprocal(invq[:], sst[:])

                    # similarity matmuls + max over support descriptors.
                    # maxv columns are (j, w) pairs: col = j*NWAY + w.
                    # DVE reduces NPAIR pair-merged PSUM tiles directly; the rest
                    # are evacuated to bf16 SBUF by ACT, then max-reduced by a
                    # DVE tensor_tensor_reduce over overlapping halves (2x mode).
                    maxv = qsm.tile([128, NCHUNK * NWAY], F32, tag="maxv")
                    for j in range(NCHUNK):
                        if USE_TTR:
                            # classes 0,1 pair-merged on DVE; 2,3,4 via ACT evac
                            # + DVE fp32 TTR over overlapping halves
                            pair = pairps.tile([128, 2, 512], F32, tag="pair")
                            for w in (0, 1):
                                nc.tensor.matmul(
                                    pair[:, w, 0:HW],
                                    lhsT=qb[C * w : C * w + C, j * 128 : (j + 1) * 128],
                                    rhs=sn[w][C * w : C * w + C, :],
                                    start=True,
                                    stop=True,
                                    tile_position=(C * w, 0),
                                )
                            nc.vector.reduce_max(
                                maxv[:, j * NWAY : j * NWAY + 2],
                                pair[:, :, 0:HW],
                                axis=AX.X,
                            )
                            for w in (2, 3, 4):
                                base = C * (w % 2)
                                sim = singps.tile([128, 512], F32, tag="sim")
                                nc.tensor.matmul(
                                    sim[:, 0:HW],
                                    lhsT=qb[base : base + C, j * 128 : (j + 1) * 128],
                                    rhs=sn[w][base : base + C, :],
                                    start=True,
                                    stop=True,
                                    tile_position=(base, 0),
                                )
                                col = j * NWAY + w
                                ev = qpool.tile([128, 448], F32, tag="ev")
                                nc.scalar.copy(ev[:, 0:HW], sim[:, 0:HW])
                                ttrash = qpool.tile([128, 224], F32, tag="ttrash")
                                nc.vector.tensor_tensor_reduce(
                                    out=ttrash[:, 0:221],
                                    in0=ev[:, 0:221],
                                    in1=ev[:, 220:441],
                                    scale=1.0,
                                    scalar=-3.0e38,
                                    op0=ALU.max,
                                    op1=ALU.max,
                                    accum_out=maxv[:, col : col + 1],
                                )
                        else:
                            # all-DVE: merged-3 block (classes 0-2) + pair (3,4);
                            # the two tiles ping-pong so PE fills one while DVE
                            # drains the other
                            m3 = pairps.tile([128, 3, 512], F32, tag="m3")
                            for w in (0, 1, 2):
                                nc.tensor.matmul(
                                    m3[:, w, 0:HW],
                                    lhsT=qb[C * (w % 2) : C * (w % 2) + C,
                                            j * 128 : (j + 1) * 128],
                                    rhs=sn[w][C * (w % 2) : C * (w % 2) + C, :],
                                    start=True,
                                    stop=True,
                                    tile_position=(C * (w % 2), 0),
                                )
                            nc.vector.reduce_max(
                                maxv[:, j * NWAY : j * NWAY + 3],
                                m3[:, :, 0:HW],
                                axis=AX.X,
                            )
                            pr = singps.tile([128, 2, 512], F32, tag="pr")
                            for w in (3, 4):
                                nc.tensor.matmul(
                                    pr[:, w - 3, 0:HW],
                                    lhsT=qb[C * (w % 2) : C * (w % 2) + C,
                                            j * 128 : (j + 1) * 128],
                                    rhs=sn[w][C * (w % 2) : C * (w % 2) + C, :],
                                    start=True,
                                    stop=True,
                                    tile_position=(C * (w % 2), 0),
                                )
                            nc.vector.reduce_max(
                                maxv[:, j * NWAY + 3 : j * NWAY + 5],
                                pr[:, :, 0:HW],
                                axis=AX.X,
                            )

                    # logits[w] = sum_j sum_m maxv[m, j*5+w] * invq[m, j]
                    # folded into 4 accumulating PE matmuls (K=128 contraction)
                    logit = logps.tile([NWAY, 1], F32, tag="logit")
                    for j in range(NCHUNK):
                        nc.tensor.matmul(
                            logit[:],
                            lhsT=maxv[:, j * NWAY : (j + 1) * NWAY],
                            rhs=invq[:, j : j + 1],
                            start=(j == 0),
                            stop=(j == NCHUNK - 1),
                            skip_group_check=True,
                        )
                    nc.vector.tensor_copy(stage[:, i : i + 1], logit[:])

            nc.sync.dma_start(out_d.rearrange("i w -> w i"), stage[:])

    nc.compile()
    return nc


def _get_program():
    if "nc" not in _CACHE:
        _CACHE["nc"] = _build_program()
    return _CACHE["nc"]


def _make_in_maps(support_xf, query_xf):
    sup = np.ascontiguousarray(np.asarray(support_xf, dtype=np.float32)).reshape(
        B, NWAY * KSHOT, C, HW
    )
    qry = np.ascontiguousarray(np.asarray(query_xf, dtype=np.float32)).reshape(B, Q, C, HW)
    idn = np.eye(128, dtype=np.float32)
    in_maps = []
    spans = []
    for core in range(8):
        bi = core // 4
        lo = (core % 4) * QPC
        hi = min(lo + QPC, Q)
        qs = qry[bi, lo:hi]
        if hi - lo < QPC:
            pad = np.repeat(qs[-1:], QPC - (hi - lo), axis=0)
            qs = np.concatenate([qs, pad], axis=0)
        in_maps.append(
            {
                "sup": np.ascontiguousarray(sup[bi]),
                "qry": np.ascontiguousarray(qs),
                "idn": idn,
            }
        )
        spans.append((bi, lo, hi))
    return in_maps, spans


def _run(in_maps, **kwargs):
    nc = _get_program()
    return run_bass_kernel_spmd(nc, in_maps, list(range(8)), **kwargs)


def kernel(support_xf, support_y, query_xf, query_y, n_way=NWAY, k_shot=KSHOT, **_):
    in_maps, spans = _make_in_maps(support_xf, query_xf)
    res = _run(in_maps)
    logits = np.zeros((B * Q, NWAY), dtype=np.float32)
    for core, (bi, lo, hi) in enumerate(spans):
        logits[bi * Q + lo : bi * Q + hi] = res.results[core]["out"][: hi - lo]
    return logits

